# revision 15
# baseline (speedup 1.0000x reference)
"""ArcMarginProduct (ArcFace) + cross-entropy loss on 8 TRN2 NeuronCores.

2x4 sharding: 2 row groups (4096 rows of x each) x 4 column shards
(~2754 of the 11014 classes each).  core = rg*4 + cs.  Per core:
  1. x rows (batches of 8 tiles): sumsq (DVE fused accum) -> rnxs =
     s/||x|| via exp(-0.5*ln(ssq)); scale-cast fp16; transpose via fp16
     PE matmuls against identity -> xT [512, 4096] fp16
  2. W col-shard (2816 padded cols, 22 tiles, batches of 8): same
     pipeline -> wnT [512, 2816] fp16 (normalized W^T)
  3. main loop, group-major for W overlap: fp16 matmuls -> PSUM
     s*cosine; ScalarE Exp with fused accum_out row-sums; DVE copy
     PSUM->SBUF f32; 8KB-chunk DMA out
  4. AllReduce (groups of 4 col-shard cores) of the partial row-sums;
     margin tail: gather s*cos[row,label] back from out (single
     multi-column indirect DMA, OOB rows skipped), compute s*phi,
     scatter into out[row,label] on the owning core, rowloss =
     (ln(S_total - 250 + e_phi - e_clab) - s*phi) * owner_flag
All ScalarE funcs (Exp/Ln/Copy) forced into one ACT table set.
Host shards inputs / assembles shards, and takes the final mean of the
8192 per-row losses (sum of flagged entries / 8192).
"""

import math
from contextlib import ExitStack

import numpy as np

# ---- problem constants (hardcoded; kernel.py must be self-contained) ----
N = 8192
D = 512
C = 11014
NCORES = 8
RGR = 2                    # row groups
CSH = 4                    # column shards
NPC = N // RGR             # 4096 rows per core
RB = NPC // 128            # 32 row blocks
CW = 2754                  # out-buffer column width per shard (uniform)
CPS = 2816                 # padded cols per shard (22 * 128)
NWT = CPS // 128           # 22 W tiles per core
TOTPAD = float(CSH * CW - C)    # 2 junk cols (shard 3) -> exp(0)=1 each
KC = D // 128              # 4 contraction chunks

S = 30.0
M = 0.6
COS_M = math.cos(M)
SIN_M = math.sin(M)
TH = math.cos(math.pi - M)
MM = math.sin(math.pi - M) * M

GW = 1024                  # main-loop group width (2 PSUM banks)
NG = (CPS + GW - 1) // GW  # 3 groups (1024, 1024, 768)
WBATCH = 8
OOB = 1 << 30

_CACHE = {}


def _patch_act_tables():
    """Force every ScalarE function we use (Exp, Ln, Copy, ...) into the
    single natural_log_exp_and_others set so bacc never inserts mid-kernel
    ACT table reloads.  Set ids are positional, so keep all entries but
    empty the others."""
    import concourse.bacc as bacc_mod

    if getattr(bacc_mod, "_arc_act_patch", False):
        return
    orig = bacc_mod.get_activation_tables

    def patched(arch):
        tabs = orig(arch)
        keep = "natural_log_exp_and_others"
        return {k: (v if k == keep else set()) for k, v in tabs.items()}

    bacc_mod.get_activation_tables = patched
    bacc_mod._arc_act_patch = True


def _patch_ldw_opt():
    """Enable walrus's LDWEIGHTS dedup pass (hardcoded off in concourse).
    Our main loop issues runs of matmuls sharing the same stationary
    operand, so deduping the per-matmul LDWEIGHTS saves PE issue slots."""
    import concourse.bass_utils as bu

    if getattr(bu, "_arc_ldw_patch", False):
        return
    orig = bu.run_command

    def patched(argv, **kwargs):
        argv = [
            "--enable-ldw-opt=true" if a == "--enable-ldw-opt=false" else a
            for a in argv
        ]
        return orig(argv, **kwargs)

    bu.run_command = patched
    bu._arc_ldw_patch = True


def _build_nc():
    import concourse.bass as bass
    import concourse.mybir as mybir
    import concourse.tile as tile
    from concourse import bacc
    import bass_rust as _br

    _patch_act_tables()

    f32 = mybir.dt.float32
    f16 = mybir.dt.float16
    bf16 = mybir.dt.bfloat16
    i32 = mybir.dt.int32
    AF = mybir.ActivationFunctionType
    Alu = mybir.AluOpType
    Ax = mybir.AxisListType

    nc = bacc.Bacc(
        "TRN2",
        target_bir_lowering=False,
        debug=False,
        num_devices=NCORES,
    )

    x_h = nc.dram_tensor("x", [NPC, D], f32, kind="ExternalInput")
    w_h = nc.dram_tensor("w", [CPS, D], f32, kind="ExternalInput")
    sidx_h = nc.dram_tensor("sidx", [128, RB], i32, kind="ExternalInput")
    flag_h = nc.dram_tensor("flag", [128, RB], f32, kind="ExternalInput")
    eye16_h = nc.dram_tensor("eye16", [128, 128], f16, kind="ExternalInput")
    out_h = nc.dram_tensor("out", [NPC * CW], f32, kind="ExternalOutput")
    rloss_h = nc.dram_tensor("rloss", [128, RB], f32, kind="ExternalOutput")

    out2d = out_h.ap().rearrange("(a b) -> a b", b=CW)
    out_flat = out_h.ap().rearrange("(a b) -> a b", b=1)

    LN_S = math.log(S)

    with tile.TileContext(nc) as tc, ExitStack() as ctx:
        const = ctx.enter_context(tc.tile_pool(name="const", bufs=1))
        xrp = ctx.enter_context(tc.tile_pool(name="xrp", bufs=WBATCH + 2))
        x16p = ctx.enter_context(tc.tile_pool(name="x16p", bufs=3))
        sqp = ctx.enter_context(tc.tile_pool(name="sqp", bufs=3))
        vec = ctx.enter_context(tc.tile_pool(name="vec", bufs=1))
        wrp = ctx.enter_context(tc.tile_pool(name="wrp", bufs=WBATCH + 2))
        w16p = ctx.enter_context(tc.tile_pool(name="w16p", bufs=3))
        big = ctx.enter_context(tc.tile_pool(name="big", bufs=1))
        ostg = ctx.enter_context(tc.tile_pool(name="ostg", bufs=4))
        escr = ctx.enter_context(tc.tile_pool(name="escr", bufs=3))
        dram = ctx.enter_context(tc.tile_pool(name="dram", bufs=1, space="DRAM"))
        psA = ctx.enter_context(tc.tile_pool(name="psA", bufs=3, space="PSUM"))
        psB = ctx.enter_context(tc.tile_pool(name="psB", bufs=2, space="PSUM"))

        # ---------------- constants / small vectors ----------------
        eye16 = const.tile([128, 128], f16)
        nc.sync.dma_start(out=eye16[:], in_=eye16_h[:])
        sidx_sb = const.tile([128, RB], i32)
        nc.sync.dma_start(out=sidx_sb[:], in_=sidx_h[:])
        flag_sb = const.tile([128, RB], f32)
        nc.sync.dma_start(out=flag_sb[:], in_=flag_h[:])
        lnS_sb = const.tile([128, 1], f32)
        nc.vector.memset(lnS_sb[:], LN_S)

        xssq = vec.tile([128, RB], f32)
        rnxs = vec.tile([128, RB], f32)
        sacc = vec.tile([128, RB * NG], f32)
        wssq = vec.tile([128, NWT], f32)
        rnw = vec.tile([128, NWT], f32)

        # ---------------- W col-shard: batches of 8 tiles ----------------
        wnT = big.tile([128, KC, CPS], f16)
        for b0 in range(0, NWT, WBATCH):
            b1 = min(b0 + WBATCH, NWT)
            wtiles = []
            for i in range(b0, b1):
                t = wrp.tile([128, D], f32, tag="wr")
                nc.sync.dma_start(out=t[:], in_=w_h[i * 128 : (i + 1) * 128, :])
                scr = sqp.tile([128, D], f32, tag="wsq")
                nc.vector.scalar_tensor_tensor(
                    out=scr[:], in0=t[:], scalar=1.0, in1=t[:],
                    op0=Alu.bypass, op1=Alu.mult,
                    accum_out=wssq[:, i : i + 1],
                )
                wtiles.append(t)
            nc.vector.tensor_scalar_max(
                out=wssq[:, b0:b1], in0=wssq[:, b0:b1], scalar1=1e-30
            )
            nc.scalar.activation(out=wssq[:, b0:b1], in_=wssq[:, b0:b1], func=AF.Ln)
            nc.scalar.activation(
                out=rnw[:, b0:b1], in_=wssq[:, b0:b1], func=AF.Exp, scale=-0.5
            )
            for j, i in enumerate(range(b0, b1)):
                w16 = w16p.tile([128, D], f16, tag="w16")
                nc.vector.tensor_scalar(
                    out=w16[:], in0=wtiles[j][:], scalar1=rnw[:, i : i + 1],
                    scalar2=None, op0=Alu.mult,
                )
                ps = psB.tile([128, D], f32)
                for k in range(KC):
                    nc.tensor.matmul(
                        out=ps[:, k * 128 : (k + 1) * 128],
                        lhsT=w16[:, k * 128 : (k + 1) * 128],
                        rhs=eye16[:],
                        start=True, stop=True,
                    )
                nc.vector.tensor_copy(
                    out=wnT[:, :, i * 128 : (i + 1) * 128],
                    in_=ps[:].rearrange("p (k c) -> p k c", k=KC),
                )

        # ---------------- X: batches of 8 tiles ----------------
        xT = big.tile([128, KC, NPC], f16)
        for b0 in range(0, RB, WBATCH):
            b1 = min(b0 + WBATCH, RB)
            xtiles = []
            for rb in range(b0, b1):
                t = xrp.tile([128, D], f32, tag="xr")
                nc.sync.dma_start(out=t[:], in_=x_h[rb * 128 : (rb + 1) * 128, :])
                scr = sqp.tile([128, D], f32, tag="sqscr")
                nc.scalar.activation(
                    out=scr[:], in_=t[:], func=AF.Square,
                    accum_out=xssq[:, rb : rb + 1],
                )
                xtiles.append(t)
            nc.vector.tensor_scalar_max(
                out=xssq[:, b0:b1], in0=xssq[:, b0:b1], scalar1=1e-30
            )
            nc.scalar.activation(out=rnxs[:, b0:b1], in_=xssq[:, b0:b1], func=AF.Ln)
            nc.scalar.activation(
                out=rnxs[:, b0:b1], in_=rnxs[:, b0:b1], func=AF.Exp,
                scale=-0.5, bias=lnS_sb[:],
            )
            for j, rb in enumerate(range(b0, b1)):
                x16 = x16p.tile([128, D], f16, tag="x16")
                nc.vector.tensor_scalar(
                    out=x16[:], in0=xtiles[j][:], scalar1=rnxs[:, rb : rb + 1],
                    scalar2=None, op0=Alu.mult,
                )
                ps = psB.tile([128, D], f32)
                for k in range(KC):
                    nc.tensor.matmul(
                        out=ps[:, k * 128 : (k + 1) * 128],
                        lhsT=x16[:, k * 128 : (k + 1) * 128],
                        rhs=eye16[:],
                        start=True, stop=True,
                    )
                nc.scalar.copy(
                    out=xT[:, :, rb * 128 : (rb + 1) * 128],
                    in_=ps[:].rearrange("p (k c) -> p k c", k=KC),
                )

        # ---------------- main loop ----------------
        out_dmas = []
        pairs = [(0, 1), (2,)]
        for pair in pairs:
            for rb in range(RB):
                pstiles = []
                for g in pair:
                    c0 = g * GW
                    gw = min(GW, CW - c0)
                    ps_mm = psA.tile([128, GW], f32, tag="mm")
                    pstiles.append((g, c0, gw, ps_mm))
                for k in range(KC):
                    for (g, c0, gw, ps) in pstiles:
                        for nb0 in range(0, gw, 512):
                            nw = min(512, gw - nb0)
                            nc.tensor.matmul(
                                out=ps[:, nb0 : nb0 + nw],
                                lhsT=xT[:, k, rb * 128 : (rb + 1) * 128],
                                rhs=wnT[:, k, c0 + nb0 : c0 + nb0 + nw],
                                start=(k == 0), stop=(k == KC - 1),
                            )
                og = ostg.tile([128, 2 * GW], f32, tag="og")
                used = 0
                dc0 = pair[0] * GW
                for (g, c0, gw, ps) in pstiles:
                    es = escr.tile([128, GW], bf16, tag="es")
                    nc.scalar.activation(
                        out=es[:, :gw], in_=ps[:, :gw], func=AF.Exp,
                        accum_out=sacc[:, rb * NG + g : rb * NG + g + 1],
                    )
                    nc.vector.tensor_copy(out=og[:, used : used + gw], in_=ps[:, :gw])
                    used += gw
                vw = max(0, min(CW - dc0, used))
                if vw > 0:
                    dma = nc.sync.dma_start(
                        out=out2d[rb * 128 : (rb + 1) * 128, dc0 : dc0 + vw],
                        in_=og[:, :vw],
                    )
                    out_dmas.append(dma)

        # ---------------- AllReduce of partial row sums ----------------
        srow = vec.tile([128, RB], f32)
        nc.vector.tensor_reduce(
            out=srow[:],
            in_=sacc[:].rearrange("p (r g) -> p r g", g=NG),
            axis=Ax.X, op=Alu.add,
        )
        ar_in = dram.tile([128, RB], f32)
        ar_out = dram.tile([CSH, 128, RB], f32)
        nc.sync.dma_start(out=ar_in[:], in_=srow[:])
        nc.gpsimd.collective_compute(
            "AllGather", Alu.bypass,
            replica_groups=[[0, 1, 2, 3], [4, 5, 6, 7]],
            ins=[ar_in.opt()],
            outs=[ar_out.opt()],
        )
        stot = vec.tile([128, RB], f32)
        spart = const.tile([128, CSH, RB], f32)
        for cs in range(CSH):
            nc.sync.dma_start(out=spart[:, cs, :], in_=ar_out[cs, :, :])
        nc.vector.tensor_tensor(
            out=stot[:], in0=spart[:, 0, :], in1=spart[:, 1, :], op=Alu.add
        )
        nc.vector.tensor_tensor(
            out=stot[:], in0=stot[:], in1=spart[:, 2, :], op=Alu.add
        )
        nc.vector.tensor_tensor(
            out=stot[:], in0=stot[:], in1=spart[:, 3, :], op=Alu.add
        )

        # ---------------- margin tail ----------------
        sclab = vec.tile([128, RB], f32)
        nc.vector.memset(sclab[:], 0.0)
        ga = nc.gpsimd.indirect_dma_start(
            out=sclab[:],
            out_offset=None,
            in_=out_flat,
            in_offset=bass.IndirectOffsetOnAxis(ap=sidx_sb[:], axis=0),
            bounds_check=NPC * CW - 1,
            oob_is_err=False,
        )
        for dma in out_dmas:
            _br.add_dep_helper(
                ga.ins, dma.ins, sync=True,
                reason="label gather after streaming out writes",
            )

        clab = vec.tile([128, RB], f32)
        nc.vector.tensor_scalar_mul(out=clab[:], in0=sclab[:], scalar1=1.0 / S)
        s2 = vec.tile([128, RB], f32)
        nc.vector.scalar_tensor_tensor(
            out=s2[:], in0=clab[:], scalar=-1.0, in1=clab[:],
            op0=Alu.mult, op1=Alu.mult,
        )
        nc.vector.tensor_scalar(
            out=s2[:], in0=s2[:], scalar1=1.0, scalar2=1e-30,
            op0=Alu.add, op1=Alu.max,
        )
        sine = vec.tile([128, RB], f32)
        nc.scalar.activation(out=sine[:], in_=s2[:], func=AF.Ln)
        nc.scalar.activation(out=sine[:], in_=sine[:], func=AF.Exp, scale=0.5)
        phi = vec.tile([128, RB], f32)
        tneg = vec.tile([128, RB], f32)
        nc.vector.tensor_scalar_mul(out=tneg[:], in0=sine[:], scalar1=-SIN_M)
        nc.vector.scalar_tensor_tensor(
            out=phi[:], in0=clab[:], scalar=COS_M, in1=tneg[:],
            op0=Alu.mult, op1=Alu.add,
        )
        mask = vec.tile([128, RB], f32)
        nc.vector.tensor_scalar(
            out=mask[:], in0=clab[:], scalar1=TH, scalar2=None, op0=Alu.is_gt
        )
        alt = vec.tile([128, RB], f32)
        nc.vector.tensor_scalar_add(out=alt[:], in0=clab[:], scalar1=-MM)
        dphi = vec.tile([128, RB], f32)
        nc.vector.tensor_tensor(out=dphi[:], in0=phi[:], in1=alt[:], op=Alu.subtract)
        nc.vector.tensor_tensor(out=dphi[:], in0=dphi[:], in1=mask[:], op=Alu.mult)
        nc.vector.tensor_tensor(out=phi[:], in0=alt[:], in1=dphi[:], op=Alu.add)
        sphi = vec.tile([128, RB], f32)
        nc.vector.tensor_scalar_mul(out=sphi[:], in0=phi[:], scalar1=S)

        # S_total = AR - 250 + flag*(e_phi - e_clab); rloss = flag*(ln(S) - s*phi)
        ephi = vec.tile([128, RB], f32)
        ecl = vec.tile([128, RB], f32)
        nc.scalar.activation(out=ephi[:], in_=sphi[:], func=AF.Exp)
        nc.scalar.activation(out=ecl[:], in_=sclab[:], func=AF.Exp)
        corr = vec.tile([128, RB], f32)
        nc.vector.tensor_tensor(out=corr[:], in0=ephi[:], in1=ecl[:], op=Alu.subtract)
        nc.vector.tensor_tensor(out=corr[:], in0=corr[:], in1=flag_sb[:], op=Alu.mult)
        nc.vector.tensor_scalar_add(out=corr[:], in0=corr[:], scalar1=-TOTPAD)
        nc.vector.tensor_tensor(out=stot[:], in0=stot[:], in1=corr[:], op=Alu.add)
        logS = vec.tile([128, RB], f32)
        nc.scalar.activation(out=logS[:], in_=stot[:], func=AF.Ln)
        rl = vec.tile([128, RB], f32)
        nc.vector.tensor_tensor(out=rl[:], in0=logS[:], in1=sphi[:], op=Alu.subtract)
        nc.vector.tensor_tensor(out=rl[:], in0=rl[:], in1=flag_sb[:], op=Alu.mult)
        nc.sync.dma_start(out=rloss_h[:], in_=rl[:])

        # scatter s*phi into out[row, label] on the owner core
        sc = nc.gpsimd.indirect_dma_start(
            out=out_flat,
            out_offset=bass.IndirectOffsetOnAxis(ap=sidx_sb[:], axis=0),
            in_=sphi[:],
            in_offset=None,
            bounds_check=NPC * CW - 1,
            oob_is_err=False,
        )
        for dma in out_dmas:
            _br.add_dep_helper(
                sc.ins, dma.ins, sync=True,
                reason="label scatter after streaming out writes",
            )

    nc.compile()
    return nc


def _get_nc():
    if "nc" not in _CACHE:
        _CACHE["nc"] = _build_nc()
    return _CACHE["nc"]


CS_START = [0, 2754, 5508, 8262]


def _make_in_maps(input, label, weight):
    x = np.ascontiguousarray(np.asarray(input, dtype=np.float32))
    lab = np.asarray(label).astype(np.int64)
    w = np.asarray(weight, dtype=np.float32)
    eye16 = np.eye(128, dtype=np.float16)
    wp_list = []
    for cs in range(CSH):
        wp = np.zeros((CPS, D), dtype=np.float32)
        s0 = CS_START[cs]
        s1 = min(s0 + CW, C)
        wp[: s1 - s0] = w[s0:s1]
        wp_list.append(wp)
    in_maps = []
    for core in range(NCORES):
        rg, cs = core // CSH, core % CSH
        r0 = rg * NPC
        lshard = lab[r0 : r0 + NPC]
        owner = (lshard // CW).astype(np.int64)
        np.minimum(owner, CSH - 1, out=owner)
        local = lshard - CS_START[cs]
        rows = np.arange(NPC, dtype=np.int64)
        sidx = np.where(owner == cs, rows * CW + local, OOB).astype(np.int32)
        flag = (owner == cs).astype(np.float32)
        # [128, RB] layout: row (rb*128 + p) -> [p, rb]
        sidx = np.ascontiguousarray(sidx.reshape(RB, 128).T)
        flag = np.ascontiguousarray(flag.reshape(RB, 128).T)
        in_maps.append(
            {
                "x": x[r0 : r0 + NPC],
                "w": wp_list[cs],
                "sidx": sidx,
                "flag": flag,
                "eye16": eye16,
            }
        )
    return in_maps


def _run(in_maps, trace=False):
    from concourse.bass_utils import run_bass_kernel_spmd

    nc = _get_nc()
    res = run_bass_kernel_spmd(
        nc, in_maps, core_ids=list(range(NCORES)), trace=trace
    )
    return res


def kernel(input, label, weight):
    in_maps = _make_in_maps(input, label, weight)
    res = _run(in_maps, trace=False)
    outs = res.results
    out = np.empty((N, C), dtype=np.float32)
    rlsum = 0.0
    for core in range(NCORES):
        rg, cs = core // CSH, core % CSH
        s0 = CS_START[cs]
        s1 = min(s0 + CW, C)
        shard = outs[core]["out"].reshape(NPC, CW)
        out[rg * NPC : (rg + 1) * NPC, s0:s1] = shard[:, : s1 - s0]
        rlsum += float(outs[core]["rloss"].sum())
    loss = np.float32(rlsum / N)
    return out, loss


# revision 16
# speedup vs baseline: 1.1269x; 1.1269x over previous
"""ArcMarginProduct (ArcFace) + cross-entropy loss on 8 TRN2 NeuronCores.

2x4 sharding: 2 row groups (4096 rows of x each) x 4 column shards
(~2754 of the 11014 classes each).  core = rg*4 + cs.  Per core:
  1. x rows (batches of 8 tiles): sumsq (DVE fused accum) -> rnxs =
     s/||x|| via exp(-0.5*ln(ssq)); scale-cast fp16; transpose via fp16
     PE matmuls against identity -> xT [512, 4096] fp16
  2. W col-shard (2816 padded cols, 22 tiles, batches of 8): same
     pipeline -> wnT [512, 2816] fp16 (normalized W^T)
  3. main loop, group-major for W overlap: fp16 matmuls -> PSUM
     s*cosine; ScalarE Exp with fused accum_out row-sums; DVE copy
     PSUM->SBUF f32; 8KB-chunk DMA out
  4. AllReduce (groups of 4 col-shard cores) of the partial row-sums;
     margin tail: gather s*cos[row,label] back from out (single
     multi-column indirect DMA, OOB rows skipped), compute s*phi,
     scatter into out[row,label] on the owning core, rowloss =
     (ln(S_total - 250 + e_phi - e_clab) - s*phi) * owner_flag
All ScalarE funcs (Exp/Ln/Copy) forced into one ACT table set.
Host shards inputs / assembles shards, and takes the final mean of the
8192 per-row losses (sum of flagged entries / 8192).
"""

import math
from contextlib import ExitStack

import numpy as np

# ---- problem constants (hardcoded; kernel.py must be self-contained) ----
N = 8192
D = 512
C = 11014
NCORES = 8
RGR = 2                    # row groups
CSH = 4                    # column shards
NPC = N // RGR             # 4096 rows per core
RB = NPC // 128            # 32 row blocks
CW = 2754                  # out-buffer column width per shard (uniform)
CPS = 2816                 # padded cols per shard (22 * 128)
NWT = CPS // 128           # 22 W tiles per core
TOTPAD = float(CSH * CW - C)    # 2 junk cols (shard 3) -> exp(0)=1 each
KC = D // 128              # 4 contraction chunks

S = 30.0
M = 0.6
COS_M = math.cos(M)
SIN_M = math.sin(M)
TH = math.cos(math.pi - M)
MM = math.sin(math.pi - M) * M

GW = 1024                  # main-loop group width (2 PSUM banks)
NG = (CPS + GW - 1) // GW  # 3 groups (1024, 1024, 768)
WBATCH = 8
OOB = 1 << 30

_CACHE = {}


def _patch_act_tables():
    """Force every ScalarE function we use (Exp, Ln, Copy, ...) into the
    single natural_log_exp_and_others set so bacc never inserts mid-kernel
    ACT table reloads.  Set ids are positional, so keep all entries but
    empty the others."""
    import concourse.bacc as bacc_mod

    if getattr(bacc_mod, "_arc_act_patch", False):
        return
    orig = bacc_mod.get_activation_tables

    def patched(arch):
        tabs = orig(arch)
        keep = "natural_log_exp_and_others"
        return {k: (v if k == keep else set()) for k, v in tabs.items()}

    bacc_mod.get_activation_tables = patched
    bacc_mod._arc_act_patch = True


def _patch_ldw_opt():
    """Enable walrus's LDWEIGHTS dedup pass (hardcoded off in concourse).
    Our main loop issues runs of matmuls sharing the same stationary
    operand, so deduping the per-matmul LDWEIGHTS saves PE issue slots."""
    import concourse.bass_utils as bu

    if getattr(bu, "_arc_ldw_patch", False):
        return
    orig = bu.run_command

    def patched(argv, **kwargs):
        argv = [
            "--enable-ldw-opt=true" if a == "--enable-ldw-opt=false" else a
            for a in argv
        ]
        return orig(argv, **kwargs)

    bu.run_command = patched
    bu._arc_ldw_patch = True


def _build_nc():
    import concourse.bass as bass
    import concourse.mybir as mybir
    import concourse.tile as tile
    from concourse import bacc
    import bass_rust as _br

    _patch_act_tables()

    f32 = mybir.dt.float32
    f16 = mybir.dt.float16
    bf16 = mybir.dt.bfloat16
    i32 = mybir.dt.int32
    AF = mybir.ActivationFunctionType
    Alu = mybir.AluOpType
    Ax = mybir.AxisListType

    nc = bacc.Bacc(
        "TRN2",
        target_bir_lowering=False,
        debug=False,
        num_devices=NCORES,
    )

    x_h = nc.dram_tensor("x", [NPC, D], f32, kind="ExternalInput")
    w_h = nc.dram_tensor("w", [CPS, D], f32, kind="ExternalInput")
    sidx_h = nc.dram_tensor("sidx", [128, RB], i32, kind="ExternalInput")
    flag_h = nc.dram_tensor("flag", [128, RB], f32, kind="ExternalInput")
    eye16_h = nc.dram_tensor("eye16", [128, 128], f16, kind="ExternalInput")
    out_h = nc.dram_tensor("out", [NPC * CW], f32, kind="ExternalOutput")
    rloss_h = nc.dram_tensor("rloss", [128, RB], f32, kind="ExternalOutput")

    out2d = out_h.ap().rearrange("(a b) -> a b", b=CW)
    out_flat = out_h.ap().rearrange("(a b) -> a b", b=1)

    LN_S = math.log(S)

    with tile.TileContext(nc) as tc, ExitStack() as ctx:
        const = ctx.enter_context(tc.tile_pool(name="const", bufs=1))
        xrp = ctx.enter_context(tc.tile_pool(name="xrp", bufs=WBATCH + 2))
        x16p = ctx.enter_context(tc.tile_pool(name="x16p", bufs=3))
        sqp = ctx.enter_context(tc.tile_pool(name="sqp", bufs=3))
        vec = ctx.enter_context(tc.tile_pool(name="vec", bufs=1))
        wrp = ctx.enter_context(tc.tile_pool(name="wrp", bufs=WBATCH + 2))
        w16p = ctx.enter_context(tc.tile_pool(name="w16p", bufs=3))
        big = ctx.enter_context(tc.tile_pool(name="big", bufs=1))
        ostg = ctx.enter_context(tc.tile_pool(name="ostg", bufs=4))
        escr = ctx.enter_context(tc.tile_pool(name="escr", bufs=3))
        dram = ctx.enter_context(tc.tile_pool(name="dram", bufs=1, space="DRAM"))
        psA = ctx.enter_context(tc.tile_pool(name="psA", bufs=3, space="PSUM"))
        psB = ctx.enter_context(tc.tile_pool(name="psB", bufs=2, space="PSUM"))

        # ---------------- constants / small vectors ----------------
        eye16 = const.tile([128, 128], f16)
        nc.sync.dma_start(out=eye16[:], in_=eye16_h[:])
        sidx_sb = const.tile([128, RB], i32)
        nc.sync.dma_start(out=sidx_sb[:], in_=sidx_h[:])
        flag_sb = const.tile([128, RB], f32)
        nc.sync.dma_start(out=flag_sb[:], in_=flag_h[:])
        lnS_sb = const.tile([128, 1], f32)
        nc.vector.memset(lnS_sb[:], LN_S)

        xssq = vec.tile([128, RB], f32)
        rnxs = vec.tile([128, RB], f32)
        sacc = vec.tile([128, RB * NG], f32)
        wssq = vec.tile([128, NWT], f32)
        rnw = vec.tile([128, NWT], f32)

        # ---------------- W col-shard: batches of 8 tiles ----------------
        wnT = big.tile([128, KC, CPS], f16)
        for b0 in range(0, NWT, WBATCH):
            b1 = min(b0 + WBATCH, NWT)
            wtiles = []
            for i in range(b0, b1):
                t = wrp.tile([128, D], f32, tag="wr")
                nc.sync.dma_start(out=t[:], in_=w_h[i * 128 : (i + 1) * 128, :])
                scr = sqp.tile([128, D], f32, tag="wsq")
                nc.vector.scalar_tensor_tensor(
                    out=scr[:], in0=t[:], scalar=1.0, in1=t[:],
                    op0=Alu.bypass, op1=Alu.mult,
                    accum_out=wssq[:, i : i + 1],
                )
                wtiles.append(t)
            nc.vector.tensor_scalar_max(
                out=wssq[:, b0:b1], in0=wssq[:, b0:b1], scalar1=1e-30
            )
            nc.scalar.activation(out=wssq[:, b0:b1], in_=wssq[:, b0:b1], func=AF.Ln)
            nc.scalar.activation(
                out=rnw[:, b0:b1], in_=wssq[:, b0:b1], func=AF.Exp, scale=-0.5
            )
            for j, i in enumerate(range(b0, b1)):
                w16 = w16p.tile([128, D], f16, tag="w16")
                nc.vector.tensor_scalar(
                    out=w16[:], in0=wtiles[j][:], scalar1=rnw[:, i : i + 1],
                    scalar2=None, op0=Alu.mult,
                )
                ps = psB.tile([128, D], f32)
                for k in range(KC):
                    nc.tensor.matmul(
                        out=ps[:, k * 128 : (k + 1) * 128],
                        lhsT=w16[:, k * 128 : (k + 1) * 128],
                        rhs=eye16[:],
                        start=True, stop=True,
                    )
                nc.vector.tensor_copy(
                    out=wnT[:, :, i * 128 : (i + 1) * 128],
                    in_=ps[:].rearrange("p (k c) -> p k c", k=KC),
                )

        # ---------------- X prep (emitted interleaved with main loop) ----
        xT = big.tile([128, KC, NPC], f16)

        def x_batch(b0):
            b1 = min(b0 + WBATCH, RB)
            xtiles = []
            for rb in range(b0, b1):
                t = xrp.tile([128, D], f32, tag="xr")
                nc.sync.dma_start(out=t[:], in_=x_h[rb * 128 : (rb + 1) * 128, :])
                scr = sqp.tile([128, D], f32, tag="sqscr")
                nc.scalar.activation(
                    out=scr[:], in_=t[:], func=AF.Square,
                    accum_out=xssq[:, rb : rb + 1],
                )
                xtiles.append(t)
            nc.vector.tensor_scalar_max(
                out=xssq[:, b0:b1], in0=xssq[:, b0:b1], scalar1=1e-30
            )
            nc.scalar.activation(out=rnxs[:, b0:b1], in_=xssq[:, b0:b1], func=AF.Ln)
            nc.scalar.activation(
                out=rnxs[:, b0:b1], in_=rnxs[:, b0:b1], func=AF.Exp,
                scale=-0.5, bias=lnS_sb[:],
            )
            for j, rb in enumerate(range(b0, b1)):
                x16 = x16p.tile([128, D], f16, tag="x16")
                nc.vector.tensor_scalar(
                    out=x16[:], in0=xtiles[j][:], scalar1=rnxs[:, rb : rb + 1],
                    scalar2=None, op0=Alu.mult,
                )
                ps = psB.tile([128, D], f32)
                for k in range(KC):
                    nc.tensor.matmul(
                        out=ps[:, k * 128 : (k + 1) * 128],
                        lhsT=x16[:, k * 128 : (k + 1) * 128],
                        rhs=eye16[:],
                        start=True, stop=True,
                    )
                nc.scalar.copy(
                    out=xT[:, :, rb * 128 : (rb + 1) * 128],
                    in_=ps[:].rearrange("p (k c) -> p k c", k=KC),
                )

        # ---------------- main loop ----------------
        out_dmas = []

        def main_unit(pair, rb):
            pstiles = []
            for g in pair:
                c0 = g * GW
                gw = min(GW, CW - c0)
                ps_mm = psA.tile([128, GW], f32, tag="mm")
                pstiles.append((g, c0, gw, ps_mm))
            for k in range(KC):
                for (g, c0, gw, ps) in pstiles:
                    for nb0 in range(0, gw, 512):
                        nw = min(512, gw - nb0)
                        nc.tensor.matmul(
                            out=ps[:, nb0 : nb0 + nw],
                            lhsT=xT[:, k, rb * 128 : (rb + 1) * 128],
                            rhs=wnT[:, k, c0 + nb0 : c0 + nb0 + nw],
                            start=(k == 0), stop=(k == KC - 1),
                        )
            og = ostg.tile([128, 2 * GW], f32, tag="og")
            used = 0
            dc0 = pair[0] * GW
            for (g, c0, gw, ps) in pstiles:
                es = escr.tile([128, GW], bf16, tag="es")
                nc.scalar.activation(
                    out=es[:, :gw], in_=ps[:, :gw], func=AF.Exp,
                    accum_out=sacc[:, rb * NG + g : rb * NG + g + 1],
                )
                nc.vector.tensor_copy(out=og[:, used : used + gw], in_=ps[:, :gw])
                used += gw
            vw = max(0, min(CW - dc0, used))
            if vw > 0:
                dma = nc.sync.dma_start(
                    out=out2d[rb * 128 : (rb + 1) * 128, dc0 : dc0 + vw],
                    in_=og[:, :vw],
                )
                out_dmas.append(dma)

        # interleave: x batch b, then pair0 over the previous batch's rbs
        x_batch(0)
        for b0 in range(WBATCH, RB, WBATCH):
            x_batch(b0)
            for rb in range(b0 - WBATCH, b0):
                main_unit((0, 1), rb)
        for rb in range(RB - WBATCH, RB):
            main_unit((0, 1), rb)
        for rb in range(RB):
            main_unit((2,), rb)

        # ---------------- AllReduce of partial row sums ----------------
        srow = vec.tile([128, RB], f32)
        nc.vector.tensor_reduce(
            out=srow[:],
            in_=sacc[:].rearrange("p (r g) -> p r g", g=NG),
            axis=Ax.X, op=Alu.add,
        )
        ar_in = dram.tile([128, RB], f32)
        ar_out = dram.tile([CSH, 128, RB], f32)
        nc.sync.dma_start(out=ar_in[:], in_=srow[:])
        nc.gpsimd.collective_compute(
            "AllGather", Alu.bypass,
            replica_groups=[[0, 1, 2, 3], [4, 5, 6, 7]],
            ins=[ar_in.opt()],
            outs=[ar_out.opt()],
        )
        stot = vec.tile([128, RB], f32)
        spart = const.tile([128, CSH, RB], f32)
        for cs in range(CSH):
            nc.sync.dma_start(out=spart[:, cs, :], in_=ar_out[cs, :, :])
        nc.vector.tensor_tensor(
            out=stot[:], in0=spart[:, 0, :], in1=spart[:, 1, :], op=Alu.add
        )
        nc.vector.tensor_tensor(
            out=stot[:], in0=stot[:], in1=spart[:, 2, :], op=Alu.add
        )
        nc.vector.tensor_tensor(
            out=stot[:], in0=stot[:], in1=spart[:, 3, :], op=Alu.add
        )

        # ---------------- margin tail ----------------
        sclab = vec.tile([128, RB], f32)
        nc.vector.memset(sclab[:], 0.0)
        ga = nc.gpsimd.indirect_dma_start(
            out=sclab[:],
            out_offset=None,
            in_=out_flat,
            in_offset=bass.IndirectOffsetOnAxis(ap=sidx_sb[:], axis=0),
            bounds_check=NPC * CW - 1,
            oob_is_err=False,
        )
        for dma in out_dmas:
            _br.add_dep_helper(
                ga.ins, dma.ins, sync=True,
                reason="label gather after streaming out writes",
            )

        clab = vec.tile([128, RB], f32)
        nc.vector.tensor_scalar_mul(out=clab[:], in0=sclab[:], scalar1=1.0 / S)
        s2 = vec.tile([128, RB], f32)
        nc.vector.scalar_tensor_tensor(
            out=s2[:], in0=clab[:], scalar=-1.0, in1=clab[:],
            op0=Alu.mult, op1=Alu.mult,
        )
        nc.vector.tensor_scalar(
            out=s2[:], in0=s2[:], scalar1=1.0, scalar2=1e-30,
            op0=Alu.add, op1=Alu.max,
        )
        sine = vec.tile([128, RB], f32)
        nc.scalar.activation(out=sine[:], in_=s2[:], func=AF.Ln)
        nc.scalar.activation(out=sine[:], in_=sine[:], func=AF.Exp, scale=0.5)
        phi = vec.tile([128, RB], f32)
        tneg = vec.tile([128, RB], f32)
        nc.vector.tensor_scalar_mul(out=tneg[:], in0=sine[:], scalar1=-SIN_M)
        nc.vector.scalar_tensor_tensor(
            out=phi[:], in0=clab[:], scalar=COS_M, in1=tneg[:],
            op0=Alu.mult, op1=Alu.add,
        )
        mask = vec.tile([128, RB], f32)
        nc.vector.tensor_scalar(
            out=mask[:], in0=clab[:], scalar1=TH, scalar2=None, op0=Alu.is_gt
        )
        alt = vec.tile([128, RB], f32)
        nc.vector.tensor_scalar_add(out=alt[:], in0=clab[:], scalar1=-MM)
        dphi = vec.tile([128, RB], f32)
        nc.vector.tensor_tensor(out=dphi[:], in0=phi[:], in1=alt[:], op=Alu.subtract)
        nc.vector.tensor_tensor(out=dphi[:], in0=dphi[:], in1=mask[:], op=Alu.mult)
        nc.vector.tensor_tensor(out=phi[:], in0=alt[:], in1=dphi[:], op=Alu.add)
        sphi = vec.tile([128, RB], f32)
        nc.vector.tensor_scalar_mul(out=sphi[:], in0=phi[:], scalar1=S)

        # S_total = AR - 250 + flag*(e_phi - e_clab); rloss = flag*(ln(S) - s*phi)
        ephi = vec.tile([128, RB], f32)
        ecl = vec.tile([128, RB], f32)
        nc.scalar.activation(out=ephi[:], in_=sphi[:], func=AF.Exp)
        nc.scalar.activation(out=ecl[:], in_=sclab[:], func=AF.Exp)
        corr = vec.tile([128, RB], f32)
        nc.vector.tensor_tensor(out=corr[:], in0=ephi[:], in1=ecl[:], op=Alu.subtract)
        nc.vector.tensor_tensor(out=corr[:], in0=corr[:], in1=flag_sb[:], op=Alu.mult)
        nc.vector.tensor_scalar_add(out=corr[:], in0=corr[:], scalar1=-TOTPAD)
        nc.vector.tensor_tensor(out=stot[:], in0=stot[:], in1=corr[:], op=Alu.add)
        logS = vec.tile([128, RB], f32)
        nc.scalar.activation(out=logS[:], in_=stot[:], func=AF.Ln)
        rl = vec.tile([128, RB], f32)
        nc.vector.tensor_tensor(out=rl[:], in0=logS[:], in1=sphi[:], op=Alu.subtract)
        nc.vector.tensor_tensor(out=rl[:], in0=rl[:], in1=flag_sb[:], op=Alu.mult)
        nc.sync.dma_start(out=rloss_h[:], in_=rl[:])

        # scatter s*phi into out[row, label] on the owner core
        sc = nc.gpsimd.indirect_dma_start(
            out=out_flat,
            out_offset=bass.IndirectOffsetOnAxis(ap=sidx_sb[:], axis=0),
            in_=sphi[:],
            in_offset=None,
            bounds_check=NPC * CW - 1,
            oob_is_err=False,
        )
        for dma in out_dmas:
            _br.add_dep_helper(
                sc.ins, dma.ins, sync=True,
                reason="label scatter after streaming out writes",
            )

    nc.compile()
    return nc


def _get_nc():
    if "nc" not in _CACHE:
        _CACHE["nc"] = _build_nc()
    return _CACHE["nc"]


CS_START = [0, 2754, 5508, 8262]


def _make_in_maps(input, label, weight):
    x = np.ascontiguousarray(np.asarray(input, dtype=np.float32))
    lab = np.asarray(label).astype(np.int64)
    w = np.asarray(weight, dtype=np.float32)
    eye16 = np.eye(128, dtype=np.float16)
    wp_list = []
    for cs in range(CSH):
        wp = np.zeros((CPS, D), dtype=np.float32)
        s0 = CS_START[cs]
        s1 = min(s0 + CW, C)
        wp[: s1 - s0] = w[s0:s1]
        wp_list.append(wp)
    in_maps = []
    for core in range(NCORES):
        rg, cs = core // CSH, core % CSH
        r0 = rg * NPC
        lshard = lab[r0 : r0 + NPC]
        owner = (lshard // CW).astype(np.int64)
        np.minimum(owner, CSH - 1, out=owner)
        local = lshard - CS_START[cs]
        rows = np.arange(NPC, dtype=np.int64)
        sidx = np.where(owner == cs, rows * CW + local, OOB).astype(np.int32)
        flag = (owner == cs).astype(np.float32)
        # [128, RB] layout: row (rb*128 + p) -> [p, rb]
        sidx = np.ascontiguousarray(sidx.reshape(RB, 128).T)
        flag = np.ascontiguousarray(flag.reshape(RB, 128).T)
        in_maps.append(
            {
                "x": x[r0 : r0 + NPC],
                "w": wp_list[cs],
                "sidx": sidx,
                "flag": flag,
                "eye16": eye16,
            }
        )
    return in_maps


def _run(in_maps, trace=False):
    from concourse.bass_utils import run_bass_kernel_spmd

    nc = _get_nc()
    res = run_bass_kernel_spmd(
        nc, in_maps, core_ids=list(range(NCORES)), trace=trace
    )
    return res


def kernel(input, label, weight):
    in_maps = _make_in_maps(input, label, weight)
    res = _run(in_maps, trace=False)
    outs = res.results
    out = np.empty((N, C), dtype=np.float32)
    rlsum = 0.0
    for core in range(NCORES):
        rg, cs = core // CSH, core % CSH
        s0 = CS_START[cs]
        s1 = min(s0 + CW, C)
        shard = outs[core]["out"].reshape(NPC, CW)
        out[rg * NPC : (rg + 1) * NPC, s0:s1] = shard[:, : s1 - s0]
        rlsum += float(outs[core]["rloss"].sum())
    loss = np.float32(rlsum / N)
    return out, loss
